# revision 18
# baseline (speedup 1.0000x reference)
"""AreaAttention Trainium2 kernel: B=8 data-parallel over 8 NeuronCores.

Reference computation (per sample, C=128 channels, N=H*W=4096 pixels):
    q = Wq@x + bq                    ('oc,bcn->bno' proper matmul)
    k = x * colsum(Wk) + bk          ('oc,bcn->bcn' keeps c: per-channel scale!)
    v = Wv@x + bv                    ('oc,bcn->bno')
    out = x + softmax(q^T k / sqrt(C)) @ v^T

Per-core design (one sample per core, no collectives):
  - q16/k16 stored [c, n] fp16; v16 stored [m, c] fp16 (PSUM accumulates fp32).
  - Scores computed TRANSPOSED: sT[m, n] = k_chunk^T @ q  (free dim 512).
  - exp(s/sqrt(C) - 4) on ScalarE, psum -> fp16 SBUF, paired into [128,2048]
    tiles (two m-chunks side by side) to halve DVE chain-add op count.
  - PV: out[c, n] += v_chunk^T @ expS  -> output directly in [c, n] layout.
  - Softmax denominator: 2 fp16 partial-sum chains over chunk-pairs, ones-matmul
    reduces partitions AND broadcasts row-sums, reciprocal, multiply, +residual.
"""
import numpy as np

C = 128
N = 4096          # 64*64
NB = 1024         # n-block span
NBLK = N // NB    # 4
MCH = N // C      # 32 m-chunks
NPAIR = MCH // 2  # 16 chunk-pairs per block
SCALE = 1.0 / np.sqrt(np.float32(C))
ESHIFT = -4.0

_cache = {}


def _build_nc():
    import concourse.tile as tile
    from concourse import bacc, mybir

    f32 = mybir.dt.float32
    f16 = mybir.dt.float16
    ADD = mybir.AluOpType.add
    MUL = mybir.AluOpType.mult
    EXP = mybir.ActivationFunctionType.Exp

    nc = bacc.Bacc("TRN2", target_bir_lowering=False)

    x_d = nc.dram_tensor("x", [C, N], f32, kind="ExternalInput")
    wqt16_d = nc.dram_tensor("wqt16", [C, C], f16, kind="ExternalInput")
    wks_d = nc.dram_tensor("wks", [C, 1], f32, kind="ExternalInput")
    wvt16_d = nc.dram_tensor("wvt16", [C, C], f16, kind="ExternalInput")
    ident16_d = nc.dram_tensor("ident16", [C, C], f16, kind="ExternalInput")
    bq_d = nc.dram_tensor("bq", [C, 1], f32, kind="ExternalInput")
    bk_d = nc.dram_tensor("bk", [C, 1], f32, kind="ExternalInput")
    bv_d = nc.dram_tensor("bv", [C, 1], f32, kind="ExternalInput")
    out_d = nc.dram_tensor("out", [C, N], f32, kind="ExternalOutput")

    with tile.TileContext(nc) as tc:
        with tc.tile_pool(name="big", bufs=1) as big, \
             tc.tile_pool(name="small", bufs=1) as small, \
             tc.tile_pool(name="es_pool", bufs=3) as es_pool, \
             tc.tile_pool(name="p_pool", bufs=4) as p_pool, \
             tc.tile_pool(name="work", bufs=2) as work, \
             tc.tile_pool(name="ps_sc", bufs=2, space="PSUM") as ps_sc, \
             tc.tile_pool(name="ps_pv", bufs=2, space="PSUM") as ps_pv:

            # ---- load inputs ----
            xfb = big.tile([C, N], f32, tag="xfb")      # x, then x + bv (residual)
            xf16 = big.tile([C, N], f16, tag="xf16")    # x fp16 (q/v proj, k build)
            q16 = big.tile([C, N], f16, tag="q16")
            k16 = big.tile([C, N], f16, tag="k16")
            v16 = big.tile([C, N], f16, tag="v16")      # chunk j at cols [128j,128j+128) = v[m, c]

            wqt16 = small.tile([C, C], f16, tag="wqt16")
            wks = small.tile([C, 1], f32, tag="wks")
            wvt16 = small.tile([C, C], f16, tag="wvt16")
            bq = small.tile([C, 1], f32, tag="bq")
            bk = small.tile([C, 1], f32, tag="bk")
            bv = small.tile([C, 1], f32, tag="bv")
            ebias = small.tile([C, 1], f32, tag="ebias")
            ones16 = small.tile([C, C], f16, tag="ones16")
            ident16 = small.tile([C, C], f16, tag="ident16")
            one11 = small.tile([1, 1], f32, tag="one11")

            nc.sync.dma_start(xfb[:], x_d[:])
            nc.gpsimd.dma_start(xf16[:], x_d[:])      # f32 -> f16 cast
            nc.sync.dma_start(wqt16[:], wqt16_d[:])
            nc.sync.dma_start(wks[:], wks_d[:])
            nc.sync.dma_start(wvt16[:], wvt16_d[:])
            nc.sync.dma_start(bq[:], bq_d[:])
            nc.sync.dma_start(bk[:], bk_d[:])
            nc.sync.dma_start(bv[:], bv_d[:])
            nc.sync.dma_start(ident16[:], ident16_d[:])
            nc.vector.memset(ebias[:], ESHIFT)
            nc.vector.memset(ones16[:], 1.0)
            nc.vector.memset(one11[:], 1.0)

            # residual base: xfb = x + bv (per-channel bias on partition dim)
            for s in range(NBLK):
                sl = slice(s * NB, (s + 1) * NB)
                nc.vector.tensor_scalar(xfb[:, sl], xfb[:, sl], bv[:], None, op0=ADD)

            # k16 = x * colsum(Wk) + bk  (fp16, single fused DVE op per span)
            for s in range(NBLK):
                sl = slice(s * NB, (s + 1) * NB)
                nc.vector.tensor_scalar(k16[:, sl], xf16[:, sl], wks[:], bk[:],
                                        op0=MUL, op1=ADD)

            # q16 = Wq @ x + bq (fp16 matmuls, lhsT = Wq^T)
            for s in range(NBLK):
                ps = ps_sc.tile([C, NB], f32, tag="sc", name=f"qps{s}")
                for h in range(2):
                    sl = slice(s * NB + h * 512, s * NB + (h + 1) * 512)
                    nc.tensor.matmul(ps[:, h * 512:(h + 1) * 512], wqt16[:], xf16[:, sl],
                                     start=True, stop=True)
                sl = slice(s * NB, (s + 1) * NB)
                nc.vector.tensor_scalar(q16[:, sl], ps[:], bq[:], None, op0=ADD)

            # v16: chunk j = (x16 chunk)^T @ Wv^T -> [m, c]; batch 8 chunks per
            # psum tile so the fp16 cast-copy runs at [128,1024] width.
            for g in range(MCH // 8):
                ps = ps_sc.tile([C, NB], f32, tag="sc", name=f"vps{g}")
                for t in range(8):
                    j = g * 8 + t
                    nc.tensor.matmul(ps[:, t * C:(t + 1) * C],
                                     xf16[:, j * C:(j + 1) * C], wvt16[:],
                                     start=True, stop=True)
                nc.vector.tensor_copy(v16[:, g * NB:(g + 1) * NB], ps[:])

            # ---- attention, one n-block (1024 q-columns) at a time ----
            for nb in range(NBLK):
                n0 = nb * NB
                pv = ps_pv.tile([C, NB], f32, tag="pv", name=f"pv{nb}")
                chains = [p_pool.tile([C, 2 * NB], f16, tag="pacc", name=f"pacc{nb}_{i}")
                          for i in range(2)]

                for jp in range(NPAIR):
                    es2 = es_pool.tile([C, 2 * NB], f16, tag="es", name=f"es{nb}_{jp}")
                    for u in range(2):
                        j = 2 * jp + u
                        ksl = slice(j * C, (j + 1) * C)
                        sc = ps_sc.tile([C, NB], f32, tag="sc", name=f"sc{nb}_{j}")
                        for h in range(2):
                            nc.tensor.matmul(sc[:, h * 512:(h + 1) * 512],
                                             k16[:, ksl],
                                             q16[:, n0 + h * 512:n0 + (h + 1) * 512],
                                             start=True, stop=True)
                        nc.scalar.activation(es2[:, u * NB:(u + 1) * NB], sc[:], EXP,
                                             bias=ebias[:], scale=float(SCALE))
                        # PV: out[c,n] += v_chunk^T @ es
                        for h in range(2):
                            nc.tensor.matmul(pv[:, h * 512:(h + 1) * 512],
                                             v16[:, ksl],
                                             es2[:, u * NB + h * 512:u * NB + (h + 1) * 512],
                                             start=(j == 0), stop=(j == MCH - 1))
                    # denominator: 2 interleaved fp16 chains over chunk-pairs
                    ch = chains[jp % 2]
                    if jp < 2:
                        nc.vector.tensor_copy(ch[:], es2[:])
                    else:
                        nc.vector.tensor_tensor(ch[:], ch[:], es2[:], op=ADD)

                tcomb = work.tile([C, 2 * NB], f16, tag="tcomb", name=f"tc{nb}")
                nc.vector.tensor_tensor(tcomb[:], chains[0][:], chains[1][:], op=ADD)

                # row-sum over partitions + both pair-halves -> rs1 [1, 1024]
                rs1 = ps_sc.tile([1, NB], f32, tag="sc", name=f"rs1{nb}")
                for h in range(2):
                    hsl = slice(h * 512, (h + 1) * 512)
                    nc.tensor.matmul(rs1[:, hsl], ones16[:, 0:1], tcomb[:, hsl],
                                     start=True, stop=False)
                    nc.tensor.matmul(rs1[:, hsl], ones16[:, 0:1],
                                     tcomb[:, NB + h * 512:NB + (h + 1) * 512],
                                     start=False, stop=True)
                rs1s = work.tile([1, NB], f32, tag="rs1s", name=f"rs1s{nb}")
                nc.vector.tensor_copy(rs1s[:], rs1[:])

                # distribute the 1024 row-sums across partitions ([128, 8]) so
                # the iterative reciprocal costs 8 elems/lane instead of 1024
                rsT = ps_sc.tile([C, 8], f32, tag="sc", name=f"rsT{nb}")
                for t in range(8):
                    nc.tensor.matmul(rsT[:, t:t + 1], rs1s[0:1, t * C:(t + 1) * C],
                                     one11[:], start=True, stop=True)
                rbT = work.tile([C, 8], f16, tag="rbT", name=f"rbT{nb}")
                with nc.allow_low_precision(reason="fp16 softmax recip, tol 2e-2"):
                    nc.vector.reciprocal(rbT[:], rsT[:])
                # transpose back: [128, 8] -> [8, 128], then K=1 broadcast
                # matmuls replicate each 128-slice across all partitions.
                rr8p = ps_sc.tile([8, C], f16, tag="sc", name=f"rr8p{nb}")
                nc.tensor.transpose(rr8p[:], rbT[:], ident16[:])
                rr8 = work.tile([8, C], f16, tag="rr8", name=f"rr8{nb}")
                nc.vector.tensor_copy(rr8[:], rr8p[:])
                rrow = work.tile([1, NB], f16, tag="rrow", name=f"rrow{nb}")
                nc.gpsimd.dma_start(rrow[0:1, :], rr8[:, :])
                rb = ps_sc.tile([C, NB], f32, tag="sc", name=f"rb{nb}")
                for t in range(8):
                    nc.tensor.matmul(rb[:, t * C:(t + 1) * C], ones16[0:1, :],
                                     rrow[0:1, t * C:(t + 1) * C], start=True, stop=True)
                rbs = work.tile([C, NB], f32, tag="rbs", name=f"rbs{nb}")
                nc.vector.tensor_copy(rbs[:], rb[:])

                # epilogue: out = pv * (1/rowsum) + (x + bv)
                ep1 = work.tile([C, NB], f32, tag="ep1", name=f"ep{nb}")
                nc.vector.tensor_tensor(ep1[:], pv[:], rbs[:], op=MUL)
                ost = work.tile([C, NB], f32, tag="ost", name=f"ost{nb}")
                nc.vector.tensor_tensor(ost[:], ep1[:], xfb[:, n0:n0 + NB], op=ADD)
                nc.sync.dma_start(out_d[:, n0:n0 + NB], ost[:])

    nc.finalize()
    return nc


def _get_nc():
    if "nc" not in _cache:
        _cache["nc"] = _build_nc()
    return _cache["nc"]


def make_in_maps(x, Wq, bq, Wk, bk, Wv, bv):
    x = np.asarray(x, dtype=np.float32)
    B = x.shape[0]
    wqt16 = np.ascontiguousarray(np.asarray(Wq, np.float32).T).astype(np.float16)
    wks = np.asarray(Wk, np.float32).sum(axis=0).reshape(C, 1)
    wvt16 = np.ascontiguousarray(np.asarray(Wv, np.float32).T).astype(np.float16)
    bq_ = np.asarray(bq, np.float32).reshape(C, 1)
    bk_ = np.asarray(bk, np.float32).reshape(C, 1)
    bv_ = np.asarray(bv, np.float32).reshape(C, 1)
    ident16 = np.eye(C, dtype=np.float16)
    in_maps = []
    for i in range(B):
        in_maps.append({
            "x": np.ascontiguousarray(x[i].reshape(C, N)),
            "wqt16": wqt16, "wks": wks, "wvt16": wvt16,
            "bq": bq_, "bk": bk_, "bv": bv_, "ident16": ident16,
        })
    return in_maps


def kernel(x, Wq, bq, Wk, bk, Wv, bv, _trace=False, _tmpdir=None):
    from concourse.bass_utils import run_bass_kernel_spmd

    x = np.asarray(x, dtype=np.float32)
    B, c, H, W = x.shape
    assert (c, H * W) == (C, N), (c, H, W)
    in_maps = make_in_maps(x, Wq, bq, Wk, bk, Wv, bv)
    nc = _get_nc()
    res = run_bass_kernel_spmd(nc, in_maps, core_ids=list(range(B)),
                               trace=_trace, tmpdir=_tmpdir)
    out = np.stack([res.results[i]["out"].reshape(C, H, W) for i in range(B)])
    if _trace:
        _cache["last_result"] = res
    return out.astype(np.float32)


# revision 20
# speedup vs baseline: 1.0838x; 1.0838x over previous
"""AreaAttention Trainium2 kernel: B=8 data-parallel over 8 NeuronCores.

Reference computation (per sample, C=128 channels, N=H*W=4096 pixels):
    q = Wq@x + bq                    ('oc,bcn->bno' proper matmul)
    k = x * colsum(Wk) + bk          ('oc,bcn->bcn' keeps c: per-channel scale!)
    v = Wv@x + bv                    ('oc,bcn->bno')
    out = x + softmax(q^T k / sqrt(C)) @ v^T

Per-core design (one sample per core, no collectives):
  - q16/k16 stored [c, n] fp16; v16 stored [m, c] fp16 (PSUM accumulates fp32).
  - Scores computed TRANSPOSED: sT[m, n] = k_chunk^T @ q  (free dim 512).
  - exp(s/sqrt(C) - 4) on ScalarE, psum -> fp16 SBUF, paired into [128,2048]
    tiles (two m-chunks side by side) to halve DVE chain-add op count.
  - PV: out[c, n] += v_chunk^T @ expS  -> output directly in [c, n] layout.
  - Softmax denominator: 2 fp16 partial-sum chains over chunk-pairs, ones-matmul
    reduces partitions AND broadcasts row-sums, reciprocal, multiply, +residual.
"""
import numpy as np

C = 128
N = 4096          # 64*64
NB = 1024         # n-block span
NBLK = N // NB    # 4
MCH = N // C      # 32 m-chunks
NPAIR = MCH // 2  # 16 chunk-pairs per block
SCALE = 1.0 / np.sqrt(np.float32(C))
ESHIFT = -4.0

_cache = {}


def _build_nc():
    import concourse.tile as tile
    from concourse import bacc, mybir

    f32 = mybir.dt.float32
    f16 = mybir.dt.float16
    ADD = mybir.AluOpType.add
    MUL = mybir.AluOpType.mult
    EXP = mybir.ActivationFunctionType.Exp

    nc = bacc.Bacc("TRN2", target_bir_lowering=False)

    x_d = nc.dram_tensor("x", [C, N], f32, kind="ExternalInput")
    wqt16_d = nc.dram_tensor("wqt16", [C, C], f16, kind="ExternalInput")
    wks_d = nc.dram_tensor("wks", [C, 1], f32, kind="ExternalInput")
    wvt16_d = nc.dram_tensor("wvt16", [C, C], f16, kind="ExternalInput")
    bq_d = nc.dram_tensor("bq", [C, 1], f32, kind="ExternalInput")
    bk_d = nc.dram_tensor("bk", [C, 1], f32, kind="ExternalInput")
    bv_d = nc.dram_tensor("bv", [C, 1], f32, kind="ExternalInput")
    out_d = nc.dram_tensor("out", [C, N], f32, kind="ExternalOutput")

    with tile.TileContext(nc) as tc:
        with tc.tile_pool(name="big", bufs=1) as big, \
             tc.tile_pool(name="small", bufs=1) as small, \
             tc.tile_pool(name="es_pool", bufs=6) as es_pool, \
             tc.tile_pool(name="p_pool", bufs=4) as p_pool, \
             tc.tile_pool(name="work", bufs=2) as work, \
             tc.tile_pool(name="ps_sc", bufs=2, space="PSUM") as ps_sc, \
             tc.tile_pool(name="ps_pv", bufs=1, space="PSUM") as ps_pv, \
             tc.tile_pool(name="ps_rs", bufs=1, space="PSUM") as ps_rs:

            # ---- load inputs ----
            xfb = big.tile([C, N], f32, tag="xfb")      # x, then x + bv (residual)
            xf16 = big.tile([C, N], f16, tag="xf16")    # x fp16 (q/v proj, k build)
            q16 = big.tile([C, N], f16, tag="q16")
            k16 = big.tile([C, N], f16, tag="k16")
            v16 = big.tile([C, N], f16, tag="v16")      # chunk j at cols [128j,128j+128) = v[m, c]

            wqt16 = small.tile([C, C], f16, tag="wqt16")
            wks = small.tile([C, 1], f32, tag="wks")
            wvt16 = small.tile([C, C], f16, tag="wvt16")
            bq = small.tile([C, 1], f32, tag="bq")
            bk = small.tile([C, 1], f32, tag="bk")
            bv = small.tile([C, 1], f32, tag="bv")
            ebias = small.tile([C, 1], f32, tag="ebias")
            ones16 = small.tile([C, C], f16, tag="ones16")

            nc.sync.dma_start(xfb[:], x_d[:])
            nc.gpsimd.dma_start(xf16[:], x_d[:])      # f32 -> f16 cast
            nc.sync.dma_start(wqt16[:], wqt16_d[:])
            nc.sync.dma_start(wks[:], wks_d[:])
            nc.sync.dma_start(wvt16[:], wvt16_d[:])
            nc.sync.dma_start(bq[:], bq_d[:])
            nc.sync.dma_start(bk[:], bk_d[:])
            nc.sync.dma_start(bv[:], bv_d[:])
            nc.vector.memset(ebias[:], ESHIFT)
            nc.vector.memset(ones16[:], 1.0)

            # residual base: xfb = x + bv (per-channel bias on partition dim)
            for s in range(NBLK):
                sl = slice(s * NB, (s + 1) * NB)
                nc.vector.tensor_scalar(xfb[:, sl], xfb[:, sl], bv[:], None, op0=ADD)

            # k16 = x * colsum(Wk) + bk  (fp16, single fused DVE op per span)
            for s in range(NBLK):
                sl = slice(s * NB, (s + 1) * NB)
                nc.vector.tensor_scalar(k16[:, sl], xf16[:, sl], wks[:], bk[:],
                                        op0=MUL, op1=ADD)

            # q16 = Wq @ x + bq (fp16 matmuls, lhsT = Wq^T)
            for s in range(NBLK):
                ps = ps_sc.tile([C, NB], f32, tag="sc", name=f"qps{s}")
                for h in range(2):
                    sl = slice(s * NB + h * 512, s * NB + (h + 1) * 512)
                    nc.tensor.matmul(ps[:, h * 512:(h + 1) * 512], wqt16[:], xf16[:, sl],
                                     start=True, stop=True)
                sl = slice(s * NB, (s + 1) * NB)
                nc.vector.tensor_scalar(q16[:, sl], ps[:], bq[:], None, op0=ADD)

            # v16: chunk j = (x16 chunk)^T @ Wv^T -> [m, c]; batch 8 chunks per
            # psum tile so the fp16 cast-copy runs at [128,1024] width.
            for g in range(MCH // 8):
                ps = ps_sc.tile([C, NB], f32, tag="sc", name=f"vps{g}")
                for t in range(8):
                    j = g * 8 + t
                    nc.tensor.matmul(ps[:, t * C:(t + 1) * C],
                                     xf16[:, j * C:(j + 1) * C], wvt16[:],
                                     start=True, stop=True)
                nc.vector.tensor_copy(v16[:, g * NB:(g + 1) * NB], ps[:])

            # ---- attention, one n-block (1024 q-columns) at a time ----
            for nb in range(NBLK):
                n0 = nb * NB
                pv = ps_pv.tile([C, NB], f32, tag="pv", name=f"pv{nb}")
                chains = [p_pool.tile([C, 2 * NB], f16, tag="pacc", name=f"pacc{nb}_{i}")
                          for i in range(2)]

                for jp in range(NPAIR):
                    es2 = es_pool.tile([C, 2 * NB], f16, tag="es", name=f"es{nb}_{jp}")
                    for u in range(2):
                        j = 2 * jp + u
                        ksl = slice(j * C, (j + 1) * C)
                        sc = ps_sc.tile([C, NB], f32, tag="sc", name=f"sc{nb}_{j}")
                        for h in range(2):
                            nc.tensor.matmul(sc[:, h * 512:(h + 1) * 512],
                                             k16[:, ksl],
                                             q16[:, n0 + h * 512:n0 + (h + 1) * 512],
                                             start=True, stop=True)
                        nc.scalar.activation(es2[:, u * NB:(u + 1) * NB], sc[:], EXP,
                                             bias=ebias[:], scale=float(SCALE))
                        # PV: out[c,n] += v_chunk^T @ es
                        for h in range(2):
                            nc.tensor.matmul(pv[:, h * 512:(h + 1) * 512],
                                             v16[:, ksl],
                                             es2[:, u * NB + h * 512:u * NB + (h + 1) * 512],
                                             start=(j == 0), stop=(j == MCH - 1))
                    # denominator: 2 interleaved fp16 chains over chunk-pairs
                    ch = chains[jp % 2]
                    if jp < 2:
                        nc.vector.tensor_copy(ch[:], es2[:])
                    else:
                        nc.vector.tensor_tensor(ch[:], ch[:], es2[:], op=ADD)

                tcomb = work.tile([C, 2 * NB], f16, tag="tcomb", name=f"tc{nb}")
                nc.vector.tensor_tensor(tcomb[:], chains[0][:], chains[1][:], op=ADD)
                # row-sum over partitions + both pair-halves, broadcast to all
                # 128 partitions by the ones[128,128] stationary operand.
                rs = ps_rs.tile([C, NB], f32, tag="rs", name=f"rs{nb}")
                for h in range(2):
                    hsl = slice(h * 512, (h + 1) * 512)
                    nc.tensor.matmul(rs[:, hsl], ones16[:], tcomb[:, hsl],
                                     start=True, stop=False)
                    nc.tensor.matmul(rs[:, hsl], ones16[:], tcomb[:, NB + h * 512:NB + (h + 1) * 512],
                                     start=False, stop=True)
                rb = work.tile([C, NB], f32, tag="rb", name=f"rb{nb}")
                nc.vector.reciprocal(rb[:], rs[:])

                # epilogue: out = pv * (1/rowsum) + (x + bv)
                ep1 = work.tile([C, NB], f32, tag="ep1", name=f"ep{nb}")
                nc.vector.tensor_tensor(ep1[:], pv[:], rb[:], op=MUL)
                ost = work.tile([C, NB], f32, tag="ost", name=f"ost{nb}")
                nc.vector.tensor_tensor(ost[:], ep1[:], xfb[:, n0:n0 + NB], op=ADD)
                nc.sync.dma_start(out_d[:, n0:n0 + NB], ost[:])

    nc.finalize()
    return nc


def _get_nc():
    if "nc" not in _cache:
        _cache["nc"] = _build_nc()
    return _cache["nc"]


def make_in_maps(x, Wq, bq, Wk, bk, Wv, bv):
    x = np.asarray(x, dtype=np.float32)
    B = x.shape[0]
    wqt16 = np.ascontiguousarray(np.asarray(Wq, np.float32).T).astype(np.float16)
    wks = np.asarray(Wk, np.float32).sum(axis=0).reshape(C, 1)
    wvt16 = np.ascontiguousarray(np.asarray(Wv, np.float32).T).astype(np.float16)
    bq_ = np.asarray(bq, np.float32).reshape(C, 1)
    bk_ = np.asarray(bk, np.float32).reshape(C, 1)
    bv_ = np.asarray(bv, np.float32).reshape(C, 1)
    in_maps = []
    for i in range(B):
        in_maps.append({
            "x": np.ascontiguousarray(x[i].reshape(C, N)),
            "wqt16": wqt16, "wks": wks, "wvt16": wvt16,
            "bq": bq_, "bk": bk_, "bv": bv_,
        })
    return in_maps


def kernel(x, Wq, bq, Wk, bk, Wv, bv, _trace=False, _tmpdir=None):
    from concourse.bass_utils import run_bass_kernel_spmd

    x = np.asarray(x, dtype=np.float32)
    B, c, H, W = x.shape
    assert (c, H * W) == (C, N), (c, H, W)
    in_maps = make_in_maps(x, Wq, bq, Wk, bk, Wv, bv)
    nc = _get_nc()
    res = run_bass_kernel_spmd(nc, in_maps, core_ids=list(range(B)),
                               trace=_trace, tmpdir=_tmpdir)
    out = np.stack([res.results[i]["out"].reshape(C, H, W) for i in range(B)])
    if _trace:
        _cache["last_result"] = res
    return out.astype(np.float32)


# revision 22
# speedup vs baseline: 1.1348x; 1.0471x over previous
"""AreaAttention Trainium2 kernel: B=8 data-parallel over 8 NeuronCores.

Reference computation (per sample, C=128 channels, N=H*W=4096 pixels):
    q = Wq@x + bq                    ('oc,bcn->bno' proper matmul)
    k = x * colsum(Wk) + bk          ('oc,bcn->bcn' keeps c: per-channel scale!)
    v = Wv@x + bv                    ('oc,bcn->bno')
    out = x + softmax(q^T k / sqrt(C)) @ v^T

Per-core design (one sample per core, no collectives):
  - q16/k16 stored [c, n] fp16; v16 stored [m, c] fp16 (PSUM accumulates fp32).
  - Scores computed TRANSPOSED: sT[m, n] = k_chunk^T @ q  (free dim 512).
  - exp(s/sqrt(C) - 4) on ScalarE, psum -> fp16 SBUF, paired into [128,2048]
    tiles (two m-chunks side by side) to halve DVE chain-add op count.
  - PV: out[c, n] += v_chunk^T @ expS  -> output directly in [c, n] layout.
  - Softmax denominator: 2 fp16 partial-sum chains over chunk-pairs, ones-matmul
    reduces partitions AND broadcasts row-sums, reciprocal, multiply, +residual.
"""
import numpy as np

C = 128
N = 4096          # 64*64
NB = 1024         # n-block span
NBLK = N // NB    # 4
MCH = N // C      # 32 m-chunks
NPAIR = MCH // 2  # 16 chunk-pairs per block
SCALE = 1.0 / np.sqrt(np.float32(C))
ESHIFT = -4.0

_cache = {}


def _build_nc():
    import concourse.tile as tile
    from concourse import bacc, mybir

    f32 = mybir.dt.float32
    f16 = mybir.dt.float16
    ADD = mybir.AluOpType.add
    MUL = mybir.AluOpType.mult
    EXP = mybir.ActivationFunctionType.Exp

    nc = bacc.Bacc("TRN2", target_bir_lowering=False)

    x_d = nc.dram_tensor("x", [C, N], f32, kind="ExternalInput")
    wqt16_d = nc.dram_tensor("wqt16", [C, C], f16, kind="ExternalInput")
    wks_d = nc.dram_tensor("wks", [C, 1], f32, kind="ExternalInput")
    wvt16_d = nc.dram_tensor("wvt16", [C, C], f16, kind="ExternalInput")
    bq_d = nc.dram_tensor("bq", [C, 1], f32, kind="ExternalInput")
    bk_d = nc.dram_tensor("bk", [C, 1], f32, kind="ExternalInput")
    bv_d = nc.dram_tensor("bv", [C, 1], f32, kind="ExternalInput")
    out_d = nc.dram_tensor("out", [C, N], f32, kind="ExternalOutput")

    with tile.TileContext(nc) as tc:
        with tc.tile_pool(name="big", bufs=1) as big, \
             tc.tile_pool(name="small", bufs=1) as small, \
             tc.tile_pool(name="es_pool", bufs=6) as es_pool, \
             tc.tile_pool(name="p_pool", bufs=4) as p_pool, \
             tc.tile_pool(name="work", bufs=2) as work, \
             tc.tile_pool(name="ps_sc", bufs=2, space="PSUM") as ps_sc, \
             tc.tile_pool(name="ps_pv", bufs=2, space="PSUM") as ps_pv:

            # ---- load inputs ----
            xfb = big.tile([C, N], f32, tag="xfb")      # x, then x + bv (residual)
            xf16 = big.tile([C, N], f16, tag="xf16")    # x fp16 (q/v proj, k build)
            q16 = big.tile([C, N], f16, tag="q16")
            k16 = big.tile([C, N], f16, tag="k16")
            v16 = big.tile([C, N], f16, tag="v16")      # chunk j at cols [128j,128j+128) = v[m, c]

            wqt16 = small.tile([C, C], f16, tag="wqt16")
            wks = small.tile([C, 1], f32, tag="wks")
            wvt16 = small.tile([C, C], f16, tag="wvt16")
            bq = small.tile([C, 1], f32, tag="bq")
            bk = small.tile([C, 1], f32, tag="bk")
            bv = small.tile([C, 1], f32, tag="bv")
            ebias = small.tile([C, 1], f32, tag="ebias")
            ones16 = small.tile([C, C], f16, tag="ones16")

            nc.sync.dma_start(xfb[:], x_d[:])
            nc.gpsimd.dma_start(xf16[:], x_d[:])      # f32 -> f16 cast
            nc.sync.dma_start(wqt16[:], wqt16_d[:])
            nc.sync.dma_start(wks[:], wks_d[:])
            nc.sync.dma_start(wvt16[:], wvt16_d[:])
            nc.sync.dma_start(bq[:], bq_d[:])
            nc.sync.dma_start(bk[:], bk_d[:])
            nc.sync.dma_start(bv[:], bv_d[:])
            nc.vector.memset(ebias[:], ESHIFT)
            nc.vector.memset(ones16[:], 1.0)

            # residual base: xfb = x + bv (per-channel bias on partition dim)
            for s in range(NBLK):
                sl = slice(s * NB, (s + 1) * NB)
                nc.vector.tensor_scalar(xfb[:, sl], xfb[:, sl], bv[:], None, op0=ADD)

            # k16 = x * colsum(Wk) + bk  (fp16, single fused DVE op per span)
            for s in range(NBLK):
                sl = slice(s * NB, (s + 1) * NB)
                nc.vector.tensor_scalar(k16[:, sl], xf16[:, sl], wks[:], bk[:],
                                        op0=MUL, op1=ADD)

            # q16 = Wq @ x + bq (fp16 matmuls, lhsT = Wq^T)
            for s in range(NBLK):
                ps = ps_sc.tile([C, NB], f32, tag="sc", name=f"qps{s}")
                for h in range(2):
                    sl = slice(s * NB + h * 512, s * NB + (h + 1) * 512)
                    nc.tensor.matmul(ps[:, h * 512:(h + 1) * 512], wqt16[:], xf16[:, sl],
                                     start=True, stop=True)
                sl = slice(s * NB, (s + 1) * NB)
                nc.vector.tensor_scalar(q16[:, sl], ps[:], bq[:], None, op0=ADD)

            # v16: chunk j = (x16 chunk)^T @ Wv^T -> [m, c]; batch 8 chunks per
            # psum tile so the fp16 cast-copy runs at [128,1024] width.
            for g in range(MCH // 8):
                ps = ps_sc.tile([C, NB], f32, tag="sc", name=f"vps{g}")
                for t in range(8):
                    j = g * 8 + t
                    nc.tensor.matmul(ps[:, t * C:(t + 1) * C],
                                     xf16[:, j * C:(j + 1) * C], wvt16[:],
                                     start=True, stop=True)
                nc.vector.tensor_copy(v16[:, g * NB:(g + 1) * NB], ps[:])

            # ---- attention, one n-block (1024 q-columns) at a time ----
            for nb in range(NBLK):
                n0 = nb * NB
                pv = ps_pv.tile([C, NB], f32, tag="pv", name=f"pv{nb}")
                chains = [p_pool.tile([C, 2 * NB], f16, tag="pacc", name=f"pacc{nb}_{i}")
                          for i in range(2)]

                for jp in range(NPAIR):
                    es2 = es_pool.tile([C, 2 * NB], f16, tag="es", name=f"es{nb}_{jp}")
                    for u in range(2):
                        j = 2 * jp + u
                        ksl = slice(j * C, (j + 1) * C)
                        sc = ps_sc.tile([C, NB], f32, tag="sc", name=f"sc{nb}_{j}")
                        for h in range(2):
                            nc.tensor.matmul(sc[:, h * 512:(h + 1) * 512],
                                             k16[:, ksl],
                                             q16[:, n0 + h * 512:n0 + (h + 1) * 512],
                                             start=True, stop=True)
                        nc.scalar.activation(es2[:, u * NB:(u + 1) * NB], sc[:], EXP,
                                             bias=ebias[:], scale=float(SCALE))
                        # PV: out[c,n] += v_chunk^T @ es
                        for h in range(2):
                            nc.tensor.matmul(pv[:, h * 512:(h + 1) * 512],
                                             v16[:, ksl],
                                             es2[:, u * NB + h * 512:u * NB + (h + 1) * 512],
                                             start=(j == 0), stop=(j == MCH - 1))
                    # denominator: 2 interleaved fp16 chains over chunk-pairs
                    ch = chains[jp % 2]
                    if jp < 2:
                        nc.vector.tensor_copy(ch[:], es2[:])
                    else:
                        nc.vector.tensor_tensor(ch[:], ch[:], es2[:], op=ADD)

                tcomb = work.tile([C, 2 * NB], f16, tag="tcomb", name=f"tc{nb}")
                nc.vector.tensor_tensor(tcomb[:], chains[0][:], chains[1][:], op=ADD)
                # row-sum over partitions + both pair-halves, broadcast to all
                # 128 partitions by the ones[128,128] stationary operand.
                # Lives briefly in a scores-pool slot, then copied to SBUF so
                # the slow reciprocal never holds a PSUM bank.
                rs = ps_sc.tile([C, NB], f32, tag="sc", name=f"rs{nb}")
                for h in range(2):
                    hsl = slice(h * 512, (h + 1) * 512)
                    nc.tensor.matmul(rs[:, hsl], ones16[:], tcomb[:, hsl],
                                     start=True, stop=False)
                    nc.tensor.matmul(rs[:, hsl], ones16[:], tcomb[:, NB + h * 512:NB + (h + 1) * 512],
                                     start=False, stop=True)
                rss = work.tile([C, NB], f32, tag="rss", name=f"rss{nb}")
                nc.vector.tensor_copy(rss[:], rs[:])
                rb = work.tile([C, NB], f32, tag="rb", name=f"rb{nb}")
                nc.vector.reciprocal(rb[:], rss[:])

                # epilogue: out = pv * (1/rowsum) + (x + bv)
                ep1 = work.tile([C, NB], f32, tag="ep1", name=f"ep{nb}")
                nc.vector.tensor_tensor(ep1[:], pv[:], rb[:], op=MUL)
                ost = work.tile([C, NB], f32, tag="ost", name=f"ost{nb}")
                nc.vector.tensor_tensor(ost[:], ep1[:], xfb[:, n0:n0 + NB], op=ADD)
                nc.sync.dma_start(out_d[:, n0:n0 + NB], ost[:])

    nc.finalize()
    return nc


def _get_nc():
    if "nc" not in _cache:
        _cache["nc"] = _build_nc()
    return _cache["nc"]


def make_in_maps(x, Wq, bq, Wk, bk, Wv, bv):
    x = np.asarray(x, dtype=np.float32)
    B = x.shape[0]
    wqt16 = np.ascontiguousarray(np.asarray(Wq, np.float32).T).astype(np.float16)
    wks = np.asarray(Wk, np.float32).sum(axis=0).reshape(C, 1)
    wvt16 = np.ascontiguousarray(np.asarray(Wv, np.float32).T).astype(np.float16)
    bq_ = np.asarray(bq, np.float32).reshape(C, 1)
    bk_ = np.asarray(bk, np.float32).reshape(C, 1)
    bv_ = np.asarray(bv, np.float32).reshape(C, 1)
    in_maps = []
    for i in range(B):
        in_maps.append({
            "x": np.ascontiguousarray(x[i].reshape(C, N)),
            "wqt16": wqt16, "wks": wks, "wvt16": wvt16,
            "bq": bq_, "bk": bk_, "bv": bv_,
        })
    return in_maps


def kernel(x, Wq, bq, Wk, bk, Wv, bv, _trace=False, _tmpdir=None):
    from concourse.bass_utils import run_bass_kernel_spmd

    x = np.asarray(x, dtype=np.float32)
    B, c, H, W = x.shape
    assert (c, H * W) == (C, N), (c, H, W)
    in_maps = make_in_maps(x, Wq, bq, Wk, bk, Wv, bv)
    nc = _get_nc()
    res = run_bass_kernel_spmd(nc, in_maps, core_ids=list(range(B)),
                               trace=_trace, tmpdir=_tmpdir)
    out = np.stack([res.results[i]["out"].reshape(C, H, W) for i in range(B)])
    if _trace:
        _cache["last_result"] = res
    return out.astype(np.float32)
